# revision 5
# baseline (speedup 1.0000x reference)
"""2-layer GCN block (gcn_norm + 2x GCNConv/gelu + global mean pool) on
8 Trainium2 NeuronCores via Bass/Tile, SPMD with a 1D node partition.

kernel(**inputs) takes the FULL inputs of nn_GCNBlock_48747878809894 and
returns the full output (tuple of two (256, 64) float32 arrays).

Design:
  - norm = dis[src]*ew*dis[dst] factorized: each core scales its owned rows
    t = (h @ W) by dis before an AllGather (halo exchange); dis[dst] is
    applied to aggregated 128-node windows afterwards. Self-loops are
    analytic: agg += t_own before the dis[dst] scale.
  - Edges bucketed by (dst core, dst 128-node window) on the host; each
    window padded to C_w chunks of 128 edges (C_w = max over cores) so all
    8 cores run a single SPMD program.
  - Per chunk: one indirect-DMA gather of 128 rows (256 B each) from the
    allgathered table ([128,1] index form — the only one the HW lowers
    correctly), then indicator matmuls accumulate
    out[128 dst, 64] += eq[128e, 128d]^T @ (ew*gath)[128e, 64] in PSUM.
  - Degrees via the same indicator matmuls against the edge-weight column;
    dis = sqrt(1/(deg+1)) (self-loop included analytically).
  - Global mean pool: indicator matmuls over two 128-graph-id windows
    accumulated in PSUM across all node windows; the host sums the 8
    per-core partials and divides by per-graph counts.
"""
import numpy as np

import concourse.bacc as bacc
import concourse.bass as bass
import concourse.mybir as mybir
import concourse.tile as tile
from concourse.masks import make_identity
from concourse.bass_utils import run_bass_kernel_spmd

F32 = mybir.dt.float32
I32 = mybir.dt.int32
AF = mybir.ActivationFunctionType
OP = mybir.AluOpType


class Cfg:
    def __init__(self, N=100000, E=1200000, D=64, G=256, K=8):
        self.N, self.E, self.D, self.G, self.K = N, E, D, G, K
        self.RPC = -(-N // K)            # rows per core
        self.W = -(-self.RPC // 128)     # node windows per core
        self.NPC = self.W * 128          # padded rows per core
        self.GW = -(-G // 128)           # graph-id windows
        self.NTOT = K * self.NPC


FULL = Cfg()


def prep_host(cfg, x, edge_index, edge_weight, batch):
    """Numpy-only sharding/index prep.

    Returns in-map arrays plus per-window chunk counts Cs (shared by all
    cores — part of the SPMD program shape)."""
    K, W, RPC, NPC, D = cfg.K, cfg.W, cfg.RPC, cfg.NPC, cfg.D
    N = cfg.N
    src = np.asarray(edge_index[0], dtype=np.int64)
    dst = np.asarray(edge_index[1], dtype=np.int64)
    ewt = np.asarray(edge_weight, dtype=np.float32)
    batch = np.asarray(batch, dtype=np.int64)
    x = np.asarray(x, dtype=np.float32)

    cd = dst // RPC
    ld = dst - cd * RPC
    wid = cd * W + (ld >> 7)
    order = np.argsort(wid, kind="stable")
    src_s, ld_s, ew_s, wid_s = src[order], ld[order], ewt[order], wid[order]

    wcounts = np.bincount(wid_s, minlength=K * W).reshape(K, W)
    Cs = np.maximum(1, (wcounts.max(axis=0) + 127) // 128).astype(np.int64)
    off = np.zeros(W + 1, dtype=np.int64)
    np.cumsum(Cs, out=off[1:])                      # chunk-column offsets
    CT = int(off[-1])                               # total chunks per core

    # flat slot of each edge: (core, window, position-within-window)
    starts = np.zeros(K * W, dtype=np.int64)
    np.cumsum(wcounts.ravel()[:-1], out=starts[1:])
    pos = np.arange(len(src_s)) - starts[wid_s]
    w_of = wid_s % W
    k_of = wid_s // W
    flat = (k_of * CT + off[w_of]) * 128 + pos      # slot c*128+p global

    srcp = np.zeros(K * CT * 128, dtype=np.int32)
    ewp = np.zeros(K * CT * 128, dtype=np.float32)
    dop = np.full(K * CT * 128, -1.0, dtype=np.float32)
    srcp[flat] = ((src_s // RPC) * NPC + (src_s % RPC)).astype(np.int32)
    ewp[flat] = ew_s
    dop[flat] = (ld_s & 127).astype(np.float32)

    def to_pm(a):     # [K*CT*128] -> [K, 128, CT]; slot index = c*128+p
        return a.reshape(K, CT, 128).transpose(0, 2, 1).copy()

    srcp, ewp, dop = to_pm(srcp), to_pm(ewp), to_pm(dop)

    bp = np.full((K, NPC), -1.0, dtype=np.float32)
    for k in range(K):
        lo, hi = k * RPC, min((k + 1) * RPC, N)
        bp[k, : hi - lo] = batch[lo:hi]
    batch_pm = bp.reshape(K, W, 128).transpose(0, 2, 1).copy()

    xp = np.zeros((K, NPC, D), dtype=np.float32)
    for k in range(K):
        lo, hi = k * RPC, min((k + 1) * RPC, N)
        xp[k, : hi - lo] = x[lo:hi]
    x_t = xp.transpose(0, 2, 1).copy()

    counts = np.bincount(batch, minlength=cfg.G).astype(np.float32)
    return x_t, srcp, ewp, dop, batch_pm, counts, tuple(int(c) for c in Cs)


def build_nc(cfg, Cs, debug=False):
    K, W, NPC, D, GW = cfg.K, cfg.W, cfg.NPC, cfg.D, cfg.GW
    NTOT = cfg.NTOT
    off = [0]
    for c in Cs:
        off.append(off[-1] + c)
    CT = off[-1]
    Cmax = max(Cs)

    nc = bacc.Bacc("TRN2", target_bir_lowering=False, debug=debug)

    x_t_d = nc.dram_tensor("x_t", [D, NPC], F32, kind="ExternalInput")
    src_d = nc.dram_tensor("srcidx", [128, CT], I32, kind="ExternalInput")
    ew_d = nc.dram_tensor("ew", [128, CT], F32, kind="ExternalInput")
    do_d = nc.dram_tensor("dstoff", [128, CT], F32, kind="ExternalInput")
    bat_d = nc.dram_tensor("batch_pm", [128, W], F32, kind="ExternalInput")
    w0_d = nc.dram_tensor("w0", [D, D], F32, kind="ExternalInput")
    w1_d = nc.dram_tensor("w1", [D, D], F32, kind="ExternalInput")
    b0_d = nc.dram_tensor("b0b", [128, D], F32, kind="ExternalInput")
    b1_d = nc.dram_tensor("b1b", [128, D], F32, kind="ExternalInput")
    iota_d = nc.dram_tensor("iota", [128, 128], F32, kind="ExternalInput")
    iotag_d = [nc.dram_tensor(f"iotag{gw}", [128, 128], F32,
                              kind="ExternalInput") for gw in range(GW)]
    pool_out = [nc.dram_tensor(f"pool{L}", [GW * 128, D], F32,
                               kind="ExternalOutput") for L in (0, 1)]

    rg = [list(range(K))]

    with tile.TileContext(nc) as tc:
        with tc.tile_pool(name="const", bufs=1) as cpool, \
             tc.tile_pool(name="state", bufs=1) as spool, \
             tc.tile_pool(name="dram", bufs=1, space="DRAM") as dpool, \
             tc.tile_pool(name="eqa_p", bufs=2) as eqa_p, \
             tc.tile_pool(name="gath_p", bufs=3) as gath_p, \
             tc.tile_pool(name="gsc_p", bufs=2) as gsc_p, \
             tc.tile_pool(name="small_p", bufs=3) as small_p, \
             tc.tile_pool(name="xT_p", bufs=2) as xT_p, \
             tc.tile_pool(name="ps_misc", bufs=2, space="PSUM") as ps_misc, \
             tc.tile_pool(name="ps_t", bufs=2, space="PSUM") as ps_t, \
             tc.tile_pool(name="ps_agg", bufs=2, space="PSUM") as ps_agg, \
             tc.tile_pool(name="ps_pool", bufs=GW, space="PSUM") as ps_pool:

            iota_t = cpool.tile([128, 128], F32, name="iota_t")
            nc.sync.dma_start(iota_t[:], iota_d[:])
            iotag_t = []
            for gw in range(GW):
                tgi = cpool.tile([128, 128], F32, name=f"iotag_t{gw}")
                nc.sync.dma_start(tgi[:], iotag_d[gw][:])
                iotag_t.append(tgi)
            wt = []
            for L, wd in enumerate((w0_d, w1_d)):
                wti = cpool.tile([D, D], F32, name=f"w_t{L}")
                nc.sync.dma_start(wti[:], wd[:])
                wt.append(wti)
            bt = []
            for L, bd in enumerate((b0_d, b1_d)):
                bti = cpool.tile([128, D], F32, name=f"b_t{L}")
                nc.sync.dma_start(bti[:], bd[:])
                bt.append(bti)
            ident = cpool.tile([128, 128], F32, name="ident")
            make_identity(nc, ident[:])

            src_all = spool.tile([128, CT], I32, name="src_all")
            nc.sync.dma_start(src_all[:], src_d[:])
            ew_all = spool.tile([128, CT], F32, name="ew_all")
            nc.sync.dma_start(ew_all[:], ew_d[:])
            do_all = spool.tile([128, CT], F32, name="do_all")
            nc.sync.dma_start(do_all[:], do_d[:])
            bat_all = spool.tile([128, W], F32, name="bat_all")
            nc.sync.dma_start(bat_all[:], bat_d[:])
            t_own = [spool.tile([128, W * D], F32, name=f"t_own{L}")
                     for L in (0, 1)]
            g_all = [spool.tile([128, W * D], F32, name=f"g_all{L}")
                     for L in (0, 1)]

            ag_in = [dpool.tile([NPC, D], F32, name=f"ag_in{L}")
                     for L in (0, 1)]
            t_full = [dpool.tile([NTOT, D], F32, name=f"t_full{L}",
                                 addr_space="Shared") for L in (0, 1)]

            # phase A (degree -> dis = sqrt(1/(deg+1))) interleaved with
            # B1(L0) (t' = dis * (x @ W0)) per window, so the L0 AllGather
            # input is ready as early as possible.
            dis_w = []
            for w in range(W):
                C = Cs[w]
                csl = slice(off[w], off[w + 1])
                eqa = eqa_p.tile([128, Cmax, 128], F32, name="eqa")
                nc.vector.tensor_tensor(
                    out=eqa[:, :C, :],
                    in0=iota_t[:].unsqueeze(1).to_broadcast([128, C, 128]),
                    in1=do_all[:, csl].unsqueeze(2).to_broadcast(
                        [128, C, 128]),
                    op=OP.is_equal)
                degp = ps_misc.tile([128, 1], F32, name="degp", tag="misc", space="PSUM")
                for c in range(C):
                    nc.tensor.matmul(
                        degp[:], lhsT=eqa[:, c, :],
                        rhs=ew_all[:, off[w] + c: off[w] + c + 1],
                        start=(c == 0), stop=(c == C - 1))
                degs = small_p.tile([128, 1], F32, name="degs")
                nc.scalar.add(degs[:], degp[:], 1.0)
                rec = small_p.tile([128, 1], F32, name="rec")
                nc.vector.reciprocal(rec[:], degs[:])
                dw = spool.tile([128, 1], F32, name=f"dis_w{w}")
                nc.scalar.sqrt(dw[:], rec[:])
                dis_w.append(dw)

                # B1 for layer 0, same window
                xT = xT_p.tile([D, 128], F32, name="xT")
                nc.sync.dma_start(xT[:], x_t_d[:, w * 128:(w + 1) * 128])
                tp = ps_t.tile([128, D], F32, name="tp", space="PSUM")
                nc.tensor.matmul(tp[:], lhsT=xT[:], rhs=wt[0][:],
                                 start=True, stop=True)
                ts = t_own[0][:, w * D:(w + 1) * D]
                nc.vector.tensor_scalar(ts, tp[:], dw[:], None, OP.mult)
                nc.sync.dma_start(ag_in[0][w * 128:(w + 1) * 128, :], ts)

            for L in (0, 1):
                # B1: t' = dis * (h @ W_L) on owned rows (layer 0 done above)
                if L == 1:
                    for w in range(W):
                        trp = ps_misc.tile([D, 128], F32, name="trp",
                                         tag="misc", space="PSUM")
                        nc.tensor.transpose(
                            trp[:], g_all[0][:, w * D:(w + 1) * D], ident[:])
                        xT = xT_p.tile([D, 128], F32, name="xT")
                        nc.scalar.copy(xT[:], trp[:])
                        tp = ps_t.tile([128, D], F32, name="tp", space="PSUM")
                        nc.tensor.matmul(tp[:], lhsT=xT[:], rhs=wt[L][:],
                                         start=True, stop=True)
                        ts = t_own[L][:, w * D:(w + 1) * D]
                        nc.vector.tensor_scalar(ts, tp[:], dis_w[w][:],
                                                None, OP.mult)
                        nc.sync.dma_start(
                            ag_in[L][w * 128:(w + 1) * 128, :], ts)

                nc.gpsimd.collective_compute(
                    "AllGather", OP.bypass,
                    ins=[ag_in[L].opt()], outs=[t_full[L].opt()],
                    replica_groups=rg)

                # B3: per-chunk gathers + indicator matmuls + postops + pool
                pps = [ps_pool.tile([128, D], F32, name=f"pps{L}_{gw}",
                                    tag="pps", space="PSUM")
                       for gw in range(GW)]
                for w in range(W):
                    C = Cs[w]
                    csl = slice(off[w], off[w + 1])
                    gath = gath_p.tile([128, Cmax * D], F32, name="gath")
                    for c in range(C):
                        col = off[w] + c
                        nc.gpsimd.indirect_dma_start(
                            out=gath[:, c * D:(c + 1) * D], out_offset=None,
                            in_=t_full[L][:],
                            in_offset=bass.IndirectOffsetOnAxis(
                                ap=src_all[:, col:col + 1], axis=0))
                    gv = gath[:, : C * D].rearrange("p (c d) -> p c d", d=D)
                    gsc = gsc_p.tile([128, Cmax, D], F32, name="gsc")
                    nc.vector.tensor_tensor(
                        out=gsc[:, :C, :], in0=gv,
                        in1=ew_all[:, csl].unsqueeze(2).to_broadcast(
                            [128, C, D]),
                        op=OP.mult)
                    eqa = eqa_p.tile([128, Cmax, 128], F32, name="eqa")
                    nc.vector.tensor_tensor(
                        out=eqa[:, :C, :],
                        in0=iota_t[:].unsqueeze(1).to_broadcast([128, C, 128]),
                        in1=do_all[:, csl].unsqueeze(2).to_broadcast(
                            [128, C, 128]),
                        op=OP.is_equal)
                    aggp = ps_agg.tile([128, D], F32, name="aggp",
                                       space="PSUM")
                    for c in range(C):
                        nc.tensor.matmul(aggp[:], lhsT=eqa[:, c, :],
                                         rhs=gsc[:, c, :],
                                         start=(c == 0), stop=(c == C - 1))
                    dsl = slice(w * D, (w + 1) * D)
                    pre = small_p.tile([128, D], F32, name="pre")
                    nc.vector.tensor_tensor(out=pre[:], in0=aggp[:],
                                            in1=t_own[L][:, dsl], op=OP.add)
                    scb = small_p.tile([128, D], F32, name="scb")
                    nc.vector.tensor_scalar(scb[:], pre[:],
                                            dis_w[w][:], None, OP.mult)
                    scb2 = small_p.tile([128, D], F32, name="scb2")
                    nc.vector.tensor_tensor(out=scb2[:], in0=scb[:],
                                            in1=bt[L][:], op=OP.add)
                    gout = g_all[L][:, dsl]
                    nc.scalar.activation(gout, scb2[:], AF.Gelu)
                    for gw in range(GW):
                        eqp = small_p.tile([128, 128], F32, name=f"eqp{gw}")
                        nc.vector.tensor_scalar(eqp[:], iotag_t[gw][:],
                                                bat_all[:, w:w + 1], None,
                                                OP.is_equal)
                        nc.tensor.matmul(pps[gw][:], lhsT=eqp[:], rhs=gout,
                                         start=(w == 0), stop=(w == W - 1))
                for gw in range(GW):
                    pok = small_p.tile([128, D], F32, name=f"pok{gw}")
                    nc.scalar.copy(pok[:], pps[gw][:])
                    nc.sync.dma_start(
                        pool_out[L][gw * 128:(gw + 1) * 128, :], pok[:])

    nc.finalize()
    return nc


_NC_CACHE = {}


def get_nc(cfg, Cs):
    key = (cfg.N, cfg.E, cfg.G, cfg.K, Cs)
    if key not in _NC_CACHE:
        _NC_CACHE[key] = build_nc(cfg, Cs)
    return _NC_CACHE[key]


def make_in_maps(cfg, x_t, srcp, ewp, dop, batch_pm, W0, b0, W1, b1):
    D = cfg.D
    b0b = np.ascontiguousarray(
        np.broadcast_to(np.asarray(b0, np.float32), (128, D)))
    b1b = np.ascontiguousarray(
        np.broadcast_to(np.asarray(b1, np.float32), (128, D)))
    iota = np.ascontiguousarray(
        np.broadcast_to(np.arange(128, dtype=np.float32), (128, 128)))
    maps = []
    for k in range(cfg.K):
        m = {
            "x_t": x_t[k], "srcidx": srcp[k], "ew": ewp[k], "dstoff": dop[k],
            "batch_pm": batch_pm[k],
            "w0": np.asarray(W0, np.float32), "w1": np.asarray(W1, np.float32),
            "b0b": b0b, "b1b": b1b, "iota": iota,
        }
        for gw in range(cfg.GW):
            m[f"iotag{gw}"] = iota + gw * 128
        maps.append(m)
    return maps


def postprocess(cfg, results, counts):
    outs = []
    denom = np.maximum(counts, 1.0).astype(np.float32)
    for L in (0, 1):
        tot = np.zeros((cfg.GW * 128, cfg.D), dtype=np.float32)
        for k in range(cfg.K):
            tot += results[k][f"pool{L}"]
        outs.append((tot[: cfg.G] / denom[:, None]).astype(np.float32))
    return tuple(outs)


def kernel(x, edge_index, edge_weight, batch, W0, b0, W1, b1):
    cfg = FULL
    x_t, srcp, ewp, dop, batch_pm, counts, Cs = prep_host(
        cfg, x, edge_index, edge_weight, batch)
    nc = get_nc(cfg, Cs)
    in_maps = make_in_maps(cfg, x_t, srcp, ewp, dop, batch_pm, W0, b0, W1, b1)
    res = run_bass_kernel_spmd(nc, in_maps, list(range(cfg.K)))
    return postprocess(cfg, res.results, counts)
